# revision 3
# baseline (speedup 1.0000x reference)
"""BiLSTM + vocab projection + log_softmax on 8 TRN2 NeuronCores. v2.

Problem: nn_BiLSTM (V=32000, T=128, B=64, E=32, H=8).
Sharding: data-parallel over batch (BL=8 per core).

Architecture (vs v1 baseline, which was ACT-bound at 938us busy):
- ONE matmul pass. Per 128-row slab, logits tiles go to PSUM; ACT exps
  them into bf16 SBUF tiles (1500/2000-col strided instructions, no
  accum_out -> no ~475ns read-accumulator aux per instr). Row sums for
  the log-sum-exp come from DVE reduce (2x on bf16) and GpSimd
  tensor_tensor accumulation, off the scalar engine.
- lse = ln(sum) via exponent-bits guess + 2 Newton steps (exp only, no
  Ln table load).
- Pass 2 needs NO matmul and NO PSUM: log_softmax = ln(ex) - lse is
  decoded from the bf16 exp values with the exponent-bits line:
  ln(y) ~= bits16(y)*(ln2/128) - (127+0.0430)*ln2  (|err| <= ~0.03).
  One DVE tensor_scalar per tile: out_bf16 = (bits16(ex) * ln2/128) +
  (-(127.043)*ln2 - lse[row]), with lse exact in f32 via the
  per-partition scalar AP. Output is all bf16, upcast on the host
  (|out| ~ 10 -> total abs err ~0.05, rel ~5e-3, gate is 2e-2).
- All 8 PSUM banks serve pass-1 ping-pong exp windows (3-slot windows
  while the scan still owns bank 7 for its gate matmuls, 4-slot after).
- Scan: tanh-only ACT (sigmoid(x) = 0.5*tanh(x/2)+0.5 folded into
  weights/chain), bf16 weights + embeddings, h written once per step
  into e_both; the projection reads h1 directly and h2 via a
  reversed-AP SBUF-to-SBUF DMA per slab (DMA is exempt from the
  partition-base alignment rules). cnew and the o-gate affine run on
  GpSimd, off the DVE critical path.
"""
import sys

sys.path.insert(0, '/opt/trn_rl_repo')

import numpy as np

V, T, B, E, H = 32000, 128, 64, 32, 8
NCORES = 8
BL = B // NCORES          # 8 batch rows per core
NR = T * BL               # 1024 (t,b) rows per core
VT = 500                  # cols per PSUM slot (one 2KB bank)
NSLAB = NR // 128         # 8 slabs of 128 rows
NTILE = V // VT           # 64 vocab tiles per slab
KP = 48                   # lhsT rows: h1 0-7, ones 8, h2 40-47
LN2 = 0.6931471805599453
KLN = LN2 / 128.0         # crude-ln slope for bf16 bits
C0 = (127.0 - 0.0430357) * LN2   # bits-line intercept, mu centers the
                                 # f - log2(1+f) ripple at +-0.0298
SLOT = 512                # psum slot stride (f32 elems per partition)

_nc_cache = {}


def _build_nc():
    if 'nc' in _nc_cache:
        return _nc_cache['nc']
    import concourse.bacc as bacc
    import concourse.mybir as mybir
    from concourse.ap import AP
    from concourse.bass import IndirectOffsetOnAxis
    from concourse.tile import TileContext
    from concourse.masks import make_identity

    f32 = mybir.dt.float32
    bf16 = mybir.dt.bfloat16
    i16 = mybir.dt.int16
    i32 = mybir.dt.int32
    AF = mybir.ActivationFunctionType
    ALU = mybir.AluOpType

    nc = bacc.Bacc("TRN2", target_bir_lowering=False, debug=False)
    x_idx = nc.dram_tensor("x_idx", [128, 16], i32, kind="ExternalInput")
    emb = nc.dram_tensor("emb", [V, E], f32, kind="ExternalInput")
    wbd = nc.dram_tensor("wbd", [80, 128], bf16, kind="ExternalInput")
    biasd = nc.dram_tensor("biasd", [128, 1], f32, kind="ExternalInput")
    wout = nc.dram_tensor("wout", [KP, V], bf16, kind="ExternalInput")
    out_b = nc.dram_tensor("out_b", [NR, V], bf16, kind="ExternalOutput")

    with TileContext(nc) as tc:
        with (
            tc.tile_pool(name="const", bufs=1) as cpool,
            tc.tile_pool(name="big", bufs=1, space="PSUM") as bigpool,
            tc.tile_pool(name="gat", bufs=2) as gpool,
            tc.tile_pool(name="scan", bufs=3) as scpool,
            tc.tile_pool(name="ex", bufs=21) as expool,
            tc.tile_pool(name="ob", bufs=2) as obpool,
            tc.tile_pool(name="proj", bufs=3) as prpool,
        ):
            # ---- constants / persistent buffers ----
            wbd_sb = cpool.tile([80, 128], bf16, tag="wbd")
            nc.sync.dma_start(wbd_sb[:, :], wbd[:, :])
            bias_sb = cpool.tile([128, 1], f32, tag="bias")
            nc.sync.dma_start(bias_sb[:, :], biasd[:, :])
            wout_sb = cpool.tile([KP, V], bf16, tag="wout")
            nc.sync.dma_start(wout_sb[:, :], wout[:, :])
            idx_sb = cpool.tile([128, 16], i32, tag="idx")
            nc.sync.dma_start(idx_sb[:, :], x_idx[:, :])
            ident = cpool.tile([128, 128], f32, tag="ident")
            make_identity(nc, ident[:, :])
            czero = cpool.tile([16, BL], f32, tag="czero")
            nc.vector.memset(czero[:, :], 0.0)
            half = cpool.tile([16, 1], f32, tag="half")
            nc.vector.memset(half[:, :], 0.5)
            e_both = cpool.tile([80, NR], bf16, tag="eboth")
            nc.vector.memset(e_both[64:80, 0:BL], 0.0)   # h1[0]=h2[127]=0

            # all 8 PSUM banks, slots = 500 cols at 512-elem stride
            ps = bigpool.tile([128, 4096], f32, tag="ps")

            def slotap(s, n=1):
                if n == 1:
                    return ps[:, SLOT * s:SLOT * s + VT]
                return AP(ps.tensor, ps.offset + SLOT * s,
                          [[4096, 128], [SLOT, n], [1, VT]])

            # ---- embedding gather + transpose into e_both (bf16); emitted
            # just-in-time per 16-step chunk so it overlaps the scan ----
            def emit_gather(c):
                for d in range(2):
                    g = gpool.tile([128, E], f32, tag="g")
                    nc.gpsimd.indirect_dma_start(
                        g[:, :], None, emb[:, :],
                        IndirectOffsetOnAxis(ap=idx_sb[:, 8 * d + c:8 * d + c + 1], axis=0),
                    )
                    pt = ps[0:E, 3584:3712]
                    nc.tensor.transpose(pt, g[:, :], ident[:, :])
                    nc.vector.tensor_copy(
                        e_both[32 * d:32 * d + 32, 128 * c:128 * c + 128], pt)

            # ---- LSTM scan ----
            # gate cols: f@0-15, i@32-47, o@64-79, C@96-111 (fwd8+bwd8 each)
            pg = ps[:, 3712:3712 + BL]   # gate preacts, in bank 7
            gather_done = [0]

            def emit_scan_step(k):
                want = min((k + 8) // 16 + 1, 8)
                while gather_done[0] < want:
                    emit_gather(gather_done[0])
                    gather_done[0] += 1
                cs = slice(k * BL, (k + 1) * BL)
                nc.tensor.matmul(pg, wbd_sb[:, :], e_both[:, cs],
                                 start=True, stop=True)
                tg = scpool.tile([112, BL], f32, tag="tg")
                nc.scalar.activation(tg[:, :], pg[0:112, :], AF.Tanh,
                                     bias=bias_sb[0:112, 0:1])
                cprev = emit_scan_step.cprev if k > 0 else czero
                u1 = scpool.tile([48, BL], f32, tag="u1")
                nc.vector.scalar_tensor_tensor(u1[32:48, :], tg[0:16, :], 1.0,
                                               cprev[0:16, :], op0=ALU.add,
                                               op1=ALU.mult)
                u2 = scpool.tile([112, BL], f32, tag="u2")
                nc.vector.tensor_tensor(u2[96:112, :], u1[32:48, :], tg[32:48, :],
                                        op=ALU.add)
                cnp = scpool.tile([16, BL], f32, tag="cnp")
                nc.vector.scalar_tensor_tensor(cnp[:, :], u2[96:112, :], 0.5,
                                               tg[96:112, :], op0=ALU.mult,
                                               op1=ALU.add)
                # cnew/w on gpsimd: off the DVE critical path
                cnew = scpool.tile([16, BL], f32, tag="cnew")
                nc.gpsimd.tensor_scalar(cnew[:, :], cnp[:, :], 0.5, None,
                                        op0=ALU.add)
                emit_scan_step.cprev = cnew
                w = scpool.tile([80, BL], f32, tag="w")
                nc.gpsimd.tensor_scalar(w[64:80, :], tg[64:80, :], 0.5, 0.5,
                                        op0=ALU.mult, op1=ALU.add)
                tht = scpool.tile([80, BL], f32, tag="tht")
                nc.scalar.activation(tht[64:80, :], cnp[:, :], AF.Tanh,
                                     bias=half[:, 0:1])
                if k < T - 1:
                    ns = slice((k + 1) * BL, (k + 2) * BL)
                    nc.vector.tensor_tensor(e_both[64:80, ns], w[64:80, :],
                                            tht[64:80, :], op=ALU.mult)

            # ---- projection ----
            ex_of = {}

            def emit_P1(j, wide):
                hb = prpool.tile([KP, 128], bf16, tag="hb")
                # base-8 accesses are illegal, so layer base-0 writes:
                # zeros everywhere, ones over 0:16 (rows 9-15 hit zero wout
                # rows), h1 over 0:8. Row 8 ends up = 1.0 (bias feature).
                nc.vector.memset(hb[0:KP, :], 0.0)
                nc.vector.memset(hb[0:16, :], 1.0)
                nc.vector.tensor_copy(hb[0:8, :],
                                      e_both[64:72, 128 * j:128 * (j + 1)])
                # h2 reversed over t: blocks (127-t)*8, t ascending. Copy
                # the whole 16-row h block (base 64, legal) into hb rows
                # 32:48 (base 32): reversed h1 lands in 32:40 where the
                # wout rows are zero, reversed h2 in 40:48 where they live.
                rev = AP(e_both.tensor,
                         e_both.offset + 64 * NR + (127 - 16 * j) * BL,
                         [[NR, 16], [-BL, 16], [1, BL]])
                nc.vector.tensor_copy(
                    AP(hb.tensor, hb.offset + 32 * 128, [[128, 16], [BL, 16], [1, BL]]),
                    rev)
                # row sums: DVE reduce (1x but cheap-ish) for 2/3 of the
                # windows, GpSimd accumulate for 1/3 to keep DVE headroom
                acc = prpool.tile([128, 4 * VT], f32, tag="acc", bufs=1)
                nc.gpsimd.memset(acc[:, :], 0.0)
                sums = prpool.tile([128, 24], f32, tag="sums")
                nsum = 0
                tiles = []
                ex_of[j] = tiles
                v = 0
                wsel = 0
                widx = 0
                while v < NTILE:
                    n = min(4 if wsel == 0 else (4 if wide else 3), NTILE - v)
                    base = 0 if wsel == 0 else 4
                    wsel ^= 1
                    for q in range(n):
                        nc.tensor.matmul(
                            slotap(base + q), hb[:, :],
                            wout_sb[:, (v + q) * VT:(v + q + 1) * VT],
                            start=True, stop=True)
                    ex = expool.tile([128, 4 * VT], bf16, tag="ex")
                    exap = AP(ex.tensor, ex.offset, [[4 * VT, 128], [VT, n], [1, VT]])
                    nc.scalar.activation(exap, slotap(base, n), AF.Exp)
                    if widx % 3 != 0 and v < NTILE - 12:
                        nc.gpsimd.tensor_tensor(
                            acc[:, 0:n * VT], acc[:, 0:n * VT], ex[:, 0:n * VT],
                            op=ALU.add)
                    else:
                        nc.vector.reduce_sum(sums[:, nsum:nsum + 1],
                                             ex[:, 0:n * VT],
                                             axis=mybir.AxisListType.X)
                        nsum += 1
                    tiles.append((ex, v, n))
                    v += n
                    widx += 1
                    yield
                nc.vector.reduce_sum(sums[:, nsum:nsum + 1], acc[:, :],
                                     axis=mybir.AxisListType.X)
                nsum += 1
                red = prpool.tile([128, 4], f32, tag="red")
                nc.vector.reduce_sum(red[:, 0:1], sums[:, 0:nsum],
                                     axis=mybir.AxisListType.X)
                red_of[j] = red

            def emit_L(j, red):
                # lse = ln(red): exponent-bits guess + two Newton steps
                # L += red*exp(-L) - 1 (exp stays in the loaded table set)
                lse = prpool.tile([128, 4], f32, tag="lse")
                nc.vector.tensor_copy(red[:, 1:2], red[:, 0:1].bitcast(i32))
                nc.vector.tensor_scalar(lse[:, 0:1], red[:, 1:2],
                                        LN2 / (1 << 23), -C0,
                                        op0=ALU.mult, op1=ALU.add)
                cur, nxt = 0, 2
                for _ in range(2):
                    e = prpool.tile([128, 1], f32, tag="nwt")
                    nc.scalar.activation(e[:, :], lse[:, cur:cur + 1], AF.Exp,
                                         scale=-1.0)
                    p = prpool.tile([128, 1], f32, tag="nwp")
                    nc.vector.tensor_tensor(p[:, :], e[:, :], red[:, 0:1], op=ALU.mult)
                    nc.vector.scalar_tensor_tensor(lse[:, nxt:nxt + 1], p[:, :], -1.0,
                                                   lse[:, cur:cur + 1], op0=ALU.add,
                                                   op1=ALU.add)
                    cur, nxt = nxt, cur
                # s1 = -C0 - lse : the bias of the crude-ln decode
                s1 = prpool.tile([128, 1], f32, tag="s1")
                nc.vector.tensor_scalar(s1[:, :], lse[:, cur:cur + 1], -1.0, -C0,
                                        op0=ALU.mult, op1=ALU.add)
                return s1

            def emit_P2(j, s1):
                # out = ln(ex) - lse ~= bits16(ex)*KLN + (-C0 - lse);
                # two windows share one staging tile -> one DMA per pair
                row0 = 128 * j
                pend = None
                for ex, v0, n in ex_of.pop(j):
                    if pend is None:
                        ob = obpool.tile([128, 8 * VT], bf16, tag="ob")
                        off = 0
                        pend = (ob, v0)
                    nc.vector.tensor_scalar(ob[:, off:off + n * VT],
                                            ex[:, 0:n * VT].bitcast(i16),
                                            KLN, s1[:, 0:1],
                                            op0=ALU.mult, op1=ALU.add)
                    off += n * VT
                    if off >= 7 * VT:
                        nc.sync.dma_start(
                            out_b[row0:row0 + 128,
                                  pend[1] * VT:pend[1] * VT + off],
                            ob[:, 0:off])
                        pend = None
                    yield
                if pend is not None:
                    nc.sync.dma_start(
                        out_b[row0:row0 + 128, pend[1] * VT:pend[1] * VT + off],
                        ob[:, 0:off])

            def drain(g):
                if g is not None:
                    for _ in g:
                        pass

            # ---- interleaved emission, middle-out slab order. P2(prev)
            # windows interleave with P1(j) windows so each P1 exp's ex-pool
            # allocation follows a P2 drain (no buffer starvation stalls) ----
            order = [3, 4, 2, 5, 1, 6, 0, 7]
            ready = {j: max(16 * j + 15, 127 - 16 * j) for j in range(NSLAB)}
            scan_done = 0
            red_of = {}
            g2 = None
            for idx, j in enumerate(order):
                while scan_done < ready[j]:
                    emit_scan_step(scan_done)
                    scan_done += 1
                g1 = emit_P1(j, scan_done >= T - 1)
                if idx >= 1:
                    pj = order[idx - 1]
                    s1 = emit_L(pj, red_of.pop(pj))
                    g2 = emit_P2(pj, s1)
                more = True
                while more:
                    more = next(g1, -1) != -1
                    if g2 is not None and next(g2, -1) == -1:
                        g2 = None
                drain(g2)
                g2 = None
            while scan_done < T - 1:
                emit_scan_step(scan_done)
                scan_done += 1
            pj = order[-1]
            s1 = emit_L(pj, red_of.pop(pj))
            drain(emit_P2(pj, s1))

    nc.finalize()
    _nc_cache['nc'] = nc
    return nc


def _host_prep(inputs):
    """Per-core input maps: weight layout prep + index sharding."""
    import ml_dtypes
    inp = {k: np.asarray(v) for k, v in inputs.items()}
    # W_bd [80, 128]: rows e1 0-31 | e2 32-63 | h1 64-71 | h2 72-79;
    # cols f@0-15, i@32-47, o@64-79, C@96-111 (fwd 8 then bwd 8 in each
    # block). f/i/o scaled by 0.5 for the tanh-based sigmoid.
    W_bd = np.zeros((80, 128), np.float32)
    bias = np.zeros((128, 1), np.float32)
    for d in range(2):
        sfx = str(d + 1)
        Wf, bf = inp['Wf' + sfx], inp['bf' + sfx]
        Wi, bi = inp['Wi' + sfx], inp['bi' + sfx]
        WC, bC = inp['WC' + sfx], inp['bC' + sfx]
        Wo, bo = inp['Wo' + sfx], inp['bo' + sfx]
        er = slice(d * 32, d * 32 + 32)
        hr = slice(64 + 8 * d, 64 + 8 * d + 8)
        for base, Wg, bg in ((0, Wf, bf), (32, Wi, bi), (64, Wo, bo)):
            cols = slice(base + 8 * d, base + 8 * d + 8)
            W_bd[er, cols] = 0.5 * np.repeat(Wg[8:40].astype(np.float32), 8, axis=1)
            W_bd[hr, cols] = 0.5 * np.repeat(Wg[0:8].astype(np.float32), 8, axis=1)
            bias[cols, 0] = 0.5 * bg[0]
        cc = slice(96 + 8 * d, 96 + 8 * d + 8)
        W_bd[er, cc] = WC[8:40]
        W_bd[hr, cc] = WC[0:8]
        bias[cc, 0] = bC
    # wout48: rows 0-7 Wout[0:8] (h1), 8 bout, 40-47 Wout[8:16] (h2)
    wout40 = np.zeros((KP, V), np.float32)
    wout40[0:8] = inp['Wout'][0:8]
    wout40[8] = inp['bout']
    wout40[40:48] = inp['Wout'][8:16]
    wout40 = wout40.astype(ml_dtypes.bfloat16)
    W_bd = W_bd.astype(ml_dtypes.bfloat16)
    emb = np.ascontiguousarray(inp['emb'].astype(np.float32))
    x = inp['x']
    in_maps = []
    for c in range(NCORES):
        xl = x[:, c * BL:(c + 1) * BL].astype(np.int32)        # [T, BL]
        fwd = xl.reshape(-1)
        rev = xl[::-1].reshape(-1)
        xi = np.concatenate([fwd.reshape(8, 128).T, rev.reshape(8, 128).T],
                            axis=1)                            # [128, 16]
        in_maps.append({
            "x_idx": np.ascontiguousarray(xi),
            "emb": emb,
            "wbd": np.ascontiguousarray(W_bd),
            "biasd": bias,
            "wout": np.ascontiguousarray(wout40),
        })
    return in_maps


def kernel(**inputs):
    from concourse.bass_utils import run_bass_kernel_spmd
    nc = _build_nc()
    in_maps = _host_prep(inputs)
    res = run_bass_kernel_spmd(nc, in_maps, list(range(NCORES)))
    out = np.empty((T, B, V), np.float32)
    for c in range(NCORES):
        out[:, c * BL:(c + 1) * BL, :] = (
            res.results[c]["out_b"].astype(np.float32).reshape(T, BL, V))
    return out


# revision 6
# speedup vs baseline: 1.0609x; 1.0609x over previous
"""BiLSTM + vocab projection + log_softmax on 8 TRN2 NeuronCores. v2.

Problem: nn_BiLSTM (V=32000, T=128, B=64, E=32, H=8).
Sharding: data-parallel over batch (BL=8 per core).

Architecture (vs v1 baseline, which was ACT-bound at 938us busy):
- ONE matmul pass. Per 128-row slab, logits tiles go to PSUM; ACT exps
  them into bf16 SBUF tiles (1500/2000-col strided instructions, no
  accum_out -> no ~475ns read-accumulator aux per instr). Row sums for
  the log-sum-exp come from DVE reduce (2x on bf16) and GpSimd
  tensor_tensor accumulation, off the scalar engine.
- lse = ln(sum) via exponent-bits guess + 2 Newton steps (exp only, no
  Ln table load).
- Pass 2 needs NO matmul and NO PSUM: log_softmax = ln(ex) - lse is
  decoded from the bf16 exp values with the exponent-bits line:
  ln(y) ~= bits16(y)*(ln2/128) - (127+0.0430)*ln2  (|err| <= ~0.03).
  One DVE tensor_scalar per tile: out_bf16 = (bits16(ex) * ln2/128) +
  (-(127.043)*ln2 - lse[row]), with lse exact in f32 via the
  per-partition scalar AP. Output is all bf16, upcast on the host
  (|out| ~ 10 -> total abs err ~0.05, rel ~5e-3, gate is 2e-2).
- All 8 PSUM banks serve pass-1 ping-pong exp windows (3-slot windows
  while the scan still owns bank 7 for its gate matmuls, 4-slot after).
- Scan: tanh-only ACT (sigmoid(x) = 0.5*tanh(x/2)+0.5 folded into
  weights/chain), bf16 weights + embeddings, h written once per step
  into e_both; the projection reads h1 directly and h2 via a
  reversed-AP SBUF-to-SBUF DMA per slab (DMA is exempt from the
  partition-base alignment rules). cnew and the o-gate affine run on
  GpSimd, off the DVE critical path.
"""
import sys

sys.path.insert(0, '/opt/trn_rl_repo')

import numpy as np

V, T, B, E, H = 32000, 128, 64, 32, 8
NCORES = 8
BL = B // NCORES          # 8 batch rows per core
NR = T * BL               # 1024 (t,b) rows per core
VT = 500                  # cols per PSUM slot (one 2KB bank)
NSLAB = NR // 128         # 8 slabs of 128 rows
NTILE = V // VT           # 64 vocab tiles per slab
KP = 48                   # lhsT rows: h1 0-7, ones 8, h2 40-47
LN2 = 0.6931471805599453
KLN = LN2 / 128.0         # crude-ln slope for bf16 bits
C0 = (127.0 - 0.0430357) * LN2   # bits-line intercept, mu centers the
                                 # f - log2(1+f) ripple at +-0.0298
SLOT = 512                # psum slot stride (f32 elems per partition)

_nc_cache = {}


def _build_nc():
    if 'nc' in _nc_cache:
        return _nc_cache['nc']
    import concourse.bacc as bacc
    import concourse.mybir as mybir
    from concourse.ap import AP
    from concourse.bass import IndirectOffsetOnAxis
    from concourse.tile import TileContext
    from concourse.masks import make_identity

    f32 = mybir.dt.float32
    bf16 = mybir.dt.bfloat16
    i16 = mybir.dt.int16
    i32 = mybir.dt.int32
    AF = mybir.ActivationFunctionType
    ALU = mybir.AluOpType

    nc = bacc.Bacc("TRN2", target_bir_lowering=False, debug=False)
    x_idx = nc.dram_tensor("x_idx", [128, 16], i32, kind="ExternalInput")
    emb = nc.dram_tensor("emb", [V, E], f32, kind="ExternalInput")
    wbd = nc.dram_tensor("wbd", [80, 128], bf16, kind="ExternalInput")
    biasd = nc.dram_tensor("biasd", [128, 1], f32, kind="ExternalInput")
    wout = nc.dram_tensor("wout", [KP, V], bf16, kind="ExternalInput")
    out_b = nc.dram_tensor("out_b", [NR, V], bf16, kind="ExternalOutput")

    with TileContext(nc) as tc:
        with (
            tc.tile_pool(name="const", bufs=1) as cpool,
            tc.tile_pool(name="big", bufs=1, space="PSUM") as bigpool,
            tc.tile_pool(name="gat", bufs=2) as gpool,
            tc.tile_pool(name="scan", bufs=3) as scpool,
            tc.tile_pool(name="ex", bufs=21) as expool,
            tc.tile_pool(name="ob", bufs=2) as obpool,
            tc.tile_pool(name="proj", bufs=3) as prpool,
        ):
            # ---- constants / persistent buffers ----
            wbd_sb = cpool.tile([80, 128], bf16, tag="wbd")
            nc.sync.dma_start(wbd_sb[:, :], wbd[:, :])
            bias_sb = cpool.tile([128, 1], f32, tag="bias")
            nc.sync.dma_start(bias_sb[:, :], biasd[:, :])
            wout_sb = cpool.tile([KP, V], bf16, tag="wout")
            nc.sync.dma_start(wout_sb[:, :], wout[:, :])
            idx_sb = cpool.tile([128, 16], i32, tag="idx")
            nc.sync.dma_start(idx_sb[:, :], x_idx[:, :])
            ident = cpool.tile([128, 128], f32, tag="ident")
            make_identity(nc, ident[:, :])
            czero = cpool.tile([16, BL], f32, tag="czero")
            nc.vector.memset(czero[:, :], 0.0)
            half = cpool.tile([16, 1], f32, tag="half")
            nc.vector.memset(half[:, :], 0.5)
            e_both = cpool.tile([80, NR], bf16, tag="eboth")
            nc.vector.memset(e_both[64:80, 0:BL], 0.0)   # h1[0]=h2[127]=0

            # all 8 PSUM banks, slots = 500 cols at 512-elem stride
            ps = bigpool.tile([128, 4096], f32, tag="ps")

            def slotap(s, n=1):
                if n == 1:
                    return ps[:, SLOT * s:SLOT * s + VT]
                return AP(ps.tensor, ps.offset + SLOT * s,
                          [[4096, 128], [SLOT, n], [1, VT]])

            # ---- embedding gather + transpose into e_both (bf16); emitted
            # just-in-time per 16-step chunk so it overlaps the scan ----
            def emit_gather(c):
                for d in range(2):
                    g = gpool.tile([128, E], f32, tag="g")
                    nc.gpsimd.indirect_dma_start(
                        g[:, :], None, emb[:, :],
                        IndirectOffsetOnAxis(ap=idx_sb[:, 8 * d + c:8 * d + c + 1], axis=0),
                    )
                    pt = ps[0:E, 3584:3712]
                    nc.tensor.transpose(pt, g[:, :], ident[:, :])
                    nc.vector.tensor_copy(
                        e_both[32 * d:32 * d + 32, 128 * c:128 * c + 128], pt)

            # ---- LSTM scan ----
            # gate cols: f@0-15, i@32-47, o@64-79, C@96-111 (fwd8+bwd8 each)
            # gate preacts live in the last 8 cols of bank 7, past slot 7's
            # 500-col tile, so the scan never blocks a 4-slot exp window
            pg = ps[:, 4084:4084 + BL]
            gather_done = [0]

            def emit_scan_step(k):
                want = min((k + 8) // 16 + 1, 8)
                while gather_done[0] < want:
                    emit_gather(gather_done[0])
                    gather_done[0] += 1
                cs = slice(k * BL, (k + 1) * BL)
                nc.tensor.matmul(pg, wbd_sb[:, :], e_both[:, cs],
                                 start=True, stop=True)
                tg = scpool.tile([112, BL], f32, tag="tg")
                nc.scalar.activation(tg[:, :], pg[0:112, :], AF.Tanh,
                                     bias=bias_sb[0:112, 0:1])
                cprev = emit_scan_step.cprev if k > 0 else czero
                u1 = scpool.tile([48, BL], f32, tag="u1")
                nc.vector.scalar_tensor_tensor(u1[32:48, :], tg[0:16, :], 1.0,
                                               cprev[0:16, :], op0=ALU.add,
                                               op1=ALU.mult)
                u2 = scpool.tile([112, BL], f32, tag="u2")
                nc.vector.tensor_tensor(u2[96:112, :], u1[32:48, :], tg[32:48, :],
                                        op=ALU.add)
                cnp = scpool.tile([16, BL], f32, tag="cnp")
                nc.vector.scalar_tensor_tensor(cnp[:, :], u2[96:112, :], 0.5,
                                               tg[96:112, :], op0=ALU.mult,
                                               op1=ALU.add)
                # cnew/w on gpsimd: off the DVE critical path
                cnew = scpool.tile([16, BL], f32, tag="cnew")
                nc.gpsimd.tensor_scalar(cnew[:, :], cnp[:, :], 0.5, None,
                                        op0=ALU.add)
                emit_scan_step.cprev = cnew
                w = scpool.tile([80, BL], f32, tag="w")
                nc.gpsimd.tensor_scalar(w[64:80, :], tg[64:80, :], 0.5, 0.5,
                                        op0=ALU.mult, op1=ALU.add)
                tht = scpool.tile([80, BL], f32, tag="tht")
                nc.scalar.activation(tht[64:80, :], cnp[:, :], AF.Tanh,
                                     bias=half[:, 0:1])
                if k < T - 1:
                    ns = slice((k + 1) * BL, (k + 2) * BL)
                    nc.vector.tensor_tensor(e_both[64:80, ns], w[64:80, :],
                                            tht[64:80, :], op=ALU.mult)

            # ---- projection ----
            ex_of = {}

            def emit_P1(j, wide):
                hb = prpool.tile([KP, 128], bf16, tag="hb")
                # base-8 accesses are illegal, so layer base-0 writes:
                # zeros everywhere, ones over 0:16 (rows 9-15 hit zero wout
                # rows), h1 over 0:8. Row 8 ends up = 1.0 (bias feature).
                nc.vector.memset(hb[0:KP, :], 0.0)
                nc.vector.memset(hb[0:16, :], 1.0)
                nc.vector.tensor_copy(hb[0:8, :],
                                      e_both[64:72, 128 * j:128 * (j + 1)])
                # h2 reversed over t: blocks (127-t)*8, t ascending. Copy
                # the whole 16-row h block (base 64, legal) into hb rows
                # 32:48 (base 32): reversed h1 lands in 32:40 where the
                # wout rows are zero, reversed h2 in 40:48 where they live.
                rev = AP(e_both.tensor,
                         e_both.offset + 64 * NR + (127 - 16 * j) * BL,
                         [[NR, 16], [-BL, 16], [1, BL]])
                nc.vector.tensor_copy(
                    AP(hb.tensor, hb.offset + 32 * 128, [[128, 16], [BL, 16], [1, BL]]),
                    rev)
                # row sums: DVE reduce (1x but cheap-ish) for 2/3 of the
                # windows, GpSimd accumulate for 1/3 to keep DVE headroom
                acc = prpool.tile([128, 4 * VT], f32, tag="acc", bufs=1)
                nc.gpsimd.memset(acc[:, :], 0.0)
                sums = prpool.tile([128, 24], f32, tag="sums")
                nsum = 0
                tiles = []
                ex_of[j] = tiles
                v = 0
                wsel = 0
                widx = 0
                while v < NTILE:
                    n = min(4, NTILE - v)
                    base = 0 if wsel == 0 else 4
                    wsel ^= 1
                    for q in range(n):
                        nc.tensor.matmul(
                            slotap(base + q), hb[:, :],
                            wout_sb[:, (v + q) * VT:(v + q + 1) * VT],
                            start=True, stop=True)
                    ex = expool.tile([128, 4 * VT], bf16, tag="ex")
                    exap = AP(ex.tensor, ex.offset, [[4 * VT, 128], [VT, n], [1, VT]])
                    nc.scalar.activation(exap, slotap(base, n), AF.Exp)
                    if widx % 3 == 2 and v < NTILE - 12:
                        nc.gpsimd.tensor_tensor(
                            acc[:, 0:n * VT], acc[:, 0:n * VT], ex[:, 0:n * VT],
                            op=ALU.add)
                    else:
                        nc.vector.reduce_sum(sums[:, nsum:nsum + 1],
                                             ex[:, 0:n * VT],
                                             axis=mybir.AxisListType.X)
                        nsum += 1
                    tiles.append((ex, v, n))
                    v += n
                    widx += 1
                    yield
                nc.vector.reduce_sum(sums[:, nsum:nsum + 1], acc[:, :],
                                     axis=mybir.AxisListType.X)
                nsum += 1
                red = prpool.tile([128, 4], f32, tag="red")
                nc.vector.reduce_sum(red[:, 0:1], sums[:, 0:nsum],
                                     axis=mybir.AxisListType.X)
                red_of[j] = red

            def emit_L(j, red):
                # lse = ln(red): exponent-bits guess + two Newton steps
                # L += red*exp(-L) - 1 (exp stays in the loaded table set)
                lse = prpool.tile([128, 4], f32, tag="lse")
                nc.vector.tensor_copy(red[:, 1:2], red[:, 0:1].bitcast(i32))
                nc.vector.tensor_scalar(lse[:, 0:1], red[:, 1:2],
                                        LN2 / (1 << 23), -C0,
                                        op0=ALU.mult, op1=ALU.add)
                cur, nxt = 0, 2
                for _ in range(2):
                    e = prpool.tile([128, 1], f32, tag="nwt")
                    nc.scalar.activation(e[:, :], lse[:, cur:cur + 1], AF.Exp,
                                         scale=-1.0)
                    p = prpool.tile([128, 1], f32, tag="nwp")
                    nc.vector.tensor_tensor(p[:, :], e[:, :], red[:, 0:1], op=ALU.mult)
                    nc.vector.scalar_tensor_tensor(lse[:, nxt:nxt + 1], p[:, :], -1.0,
                                                   lse[:, cur:cur + 1], op0=ALU.add,
                                                   op1=ALU.add)
                    cur, nxt = nxt, cur
                # s1 = -C0 - lse : the bias of the crude-ln decode
                s1 = prpool.tile([128, 1], f32, tag="s1")
                nc.vector.tensor_scalar(s1[:, :], lse[:, cur:cur + 1], -1.0, -C0,
                                        op0=ALU.mult, op1=ALU.add)
                return s1

            def emit_P2(j, s1):
                # out = ln(ex) - lse ~= bits16(ex)*KLN + (-C0 - lse);
                # two windows share one staging tile -> one DMA per pair
                row0 = 128 * j
                pend = None
                for ex, v0, n in ex_of.pop(j):
                    if pend is None:
                        ob = obpool.tile([128, 8 * VT], bf16, tag="ob")
                        off = 0
                        pend = (ob, v0)
                    nc.vector.tensor_scalar(ob[:, off:off + n * VT],
                                            ex[:, 0:n * VT].bitcast(i16),
                                            KLN, s1[:, 0:1],
                                            op0=ALU.mult, op1=ALU.add)
                    off += n * VT
                    if off >= 7 * VT:
                        nc.sync.dma_start(
                            out_b[row0:row0 + 128,
                                  pend[1] * VT:pend[1] * VT + off],
                            ob[:, 0:off])
                        pend = None
                    yield
                if pend is not None:
                    nc.sync.dma_start(
                        out_b[row0:row0 + 128, pend[1] * VT:pend[1] * VT + off],
                        ob[:, 0:off])

            def drain(g):
                if g is not None:
                    for _ in g:
                        pass

            # ---- interleaved emission, middle-out slab order. P2(prev)
            # windows interleave with P1(j) windows so each P1 exp's ex-pool
            # allocation follows a P2 drain (no buffer starvation stalls) ----
            order = [3, 4, 2, 5, 1, 6, 0, 7]
            ready = {j: max(16 * j + 15, 127 - 16 * j) for j in range(NSLAB)}
            scan_done = 0
            red_of = {}
            g2 = None
            for idx, j in enumerate(order):
                while scan_done < ready[j]:
                    emit_scan_step(scan_done)
                    scan_done += 1
                g1 = emit_P1(j, scan_done >= T - 1)
                if idx >= 1:
                    pj = order[idx - 1]
                    s1 = emit_L(pj, red_of.pop(pj))
                    g2 = emit_P2(pj, s1)
                more = True
                while more:
                    more = next(g1, -1) != -1
                    if g2 is not None and next(g2, -1) == -1:
                        g2 = None
                drain(g2)
                g2 = None
            while scan_done < T - 1:
                emit_scan_step(scan_done)
                scan_done += 1
            pj = order[-1]
            s1 = emit_L(pj, red_of.pop(pj))
            drain(emit_P2(pj, s1))

    nc.finalize()
    _nc_cache['nc'] = nc
    return nc


def _host_prep(inputs):
    """Per-core input maps: weight layout prep + index sharding."""
    import ml_dtypes
    inp = {k: np.asarray(v) for k, v in inputs.items()}
    # W_bd [80, 128]: rows e1 0-31 | e2 32-63 | h1 64-71 | h2 72-79;
    # cols f@0-15, i@32-47, o@64-79, C@96-111 (fwd 8 then bwd 8 in each
    # block). f/i/o scaled by 0.5 for the tanh-based sigmoid.
    W_bd = np.zeros((80, 128), np.float32)
    bias = np.zeros((128, 1), np.float32)
    for d in range(2):
        sfx = str(d + 1)
        Wf, bf = inp['Wf' + sfx], inp['bf' + sfx]
        Wi, bi = inp['Wi' + sfx], inp['bi' + sfx]
        WC, bC = inp['WC' + sfx], inp['bC' + sfx]
        Wo, bo = inp['Wo' + sfx], inp['bo' + sfx]
        er = slice(d * 32, d * 32 + 32)
        hr = slice(64 + 8 * d, 64 + 8 * d + 8)
        for base, Wg, bg in ((0, Wf, bf), (32, Wi, bi), (64, Wo, bo)):
            cols = slice(base + 8 * d, base + 8 * d + 8)
            W_bd[er, cols] = 0.5 * np.repeat(Wg[8:40].astype(np.float32), 8, axis=1)
            W_bd[hr, cols] = 0.5 * np.repeat(Wg[0:8].astype(np.float32), 8, axis=1)
            bias[cols, 0] = 0.5 * bg[0]
        cc = slice(96 + 8 * d, 96 + 8 * d + 8)
        W_bd[er, cc] = WC[8:40]
        W_bd[hr, cc] = WC[0:8]
        bias[cc, 0] = bC
    # wout48: rows 0-7 Wout[0:8] (h1), 8 bout, 40-47 Wout[8:16] (h2)
    wout40 = np.zeros((KP, V), np.float32)
    wout40[0:8] = inp['Wout'][0:8]
    wout40[8] = inp['bout']
    wout40[40:48] = inp['Wout'][8:16]
    wout40 = wout40.astype(ml_dtypes.bfloat16)
    W_bd = W_bd.astype(ml_dtypes.bfloat16)
    emb = np.ascontiguousarray(inp['emb'].astype(np.float32))
    x = inp['x']
    in_maps = []
    for c in range(NCORES):
        xl = x[:, c * BL:(c + 1) * BL].astype(np.int32)        # [T, BL]
        fwd = xl.reshape(-1)
        rev = xl[::-1].reshape(-1)
        xi = np.concatenate([fwd.reshape(8, 128).T, rev.reshape(8, 128).T],
                            axis=1)                            # [128, 16]
        in_maps.append({
            "x_idx": np.ascontiguousarray(xi),
            "emb": emb,
            "wbd": np.ascontiguousarray(W_bd),
            "biasd": bias,
            "wout": np.ascontiguousarray(wout40),
        })
    return in_maps


def kernel(**inputs):
    from concourse.bass_utils import run_bass_kernel_spmd
    nc = _build_nc()
    in_maps = _host_prep(inputs)
    res = run_bass_kernel_spmd(nc, in_maps, list(range(NCORES)))
    out = np.empty((T, B, V), np.float32)
    for c in range(NCORES):
        out[:, c * BL:(c + 1) * BL, :] = (
            res.results[c]["out_b"].astype(np.float32).reshape(T, BL, V))
    return out


# revision 7
# speedup vs baseline: 1.0754x; 1.0136x over previous
"""BiLSTM + vocab projection + log_softmax on 8 TRN2 NeuronCores. v2.

Problem: nn_BiLSTM (V=32000, T=128, B=64, E=32, H=8).
Sharding: data-parallel over batch (BL=8 per core).

Architecture (vs v1 baseline, which was ACT-bound at 938us busy):
- ONE matmul pass. Per 128-row slab, logits tiles go to PSUM; ACT exps
  them into bf16 SBUF tiles (1500/2000-col strided instructions, no
  accum_out -> no ~475ns read-accumulator aux per instr). Row sums for
  the log-sum-exp come from DVE reduce (2x on bf16) and GpSimd
  tensor_tensor accumulation, off the scalar engine.
- lse = ln(sum) via exponent-bits guess + 2 Newton steps (exp only, no
  Ln table load).
- Pass 2 needs NO matmul and NO PSUM: log_softmax = ln(ex) - lse is
  decoded from the bf16 exp values with the exponent-bits line:
  ln(y) ~= bits16(y)*(ln2/128) - (127+0.0430)*ln2  (|err| <= ~0.03).
  One DVE tensor_scalar per tile: out_bf16 = (bits16(ex) * ln2/128) +
  (-(127.043)*ln2 - lse[row]), with lse exact in f32 via the
  per-partition scalar AP. Output is all bf16, upcast on the host
  (|out| ~ 10 -> total abs err ~0.05, rel ~5e-3, gate is 2e-2).
- All 8 PSUM banks serve pass-1 ping-pong exp windows (3-slot windows
  while the scan still owns bank 7 for its gate matmuls, 4-slot after).
- Scan: tanh-only ACT (sigmoid(x) = 0.5*tanh(x/2)+0.5 folded into
  weights/chain), bf16 weights + embeddings, h written once per step
  into e_both; the projection reads h1 directly and h2 via a
  reversed-AP SBUF-to-SBUF DMA per slab (DMA is exempt from the
  partition-base alignment rules). cnew and the o-gate affine run on
  GpSimd, off the DVE critical path.
"""
import sys

sys.path.insert(0, '/opt/trn_rl_repo')

import numpy as np

V, T, B, E, H = 32000, 128, 64, 32, 8
NCORES = 8
BL = B // NCORES          # 8 batch rows per core
NR = T * BL               # 1024 (t,b) rows per core
VT = 500                  # cols per PSUM slot (one 2KB bank)
NSLAB = NR // 128         # 8 slabs of 128 rows
NTILE = V // VT           # 64 vocab tiles per slab
KP = 48                   # lhsT rows: h1 0-7, ones 8, h2 40-47
LN2 = 0.6931471805599453
KLN = LN2 / 128.0         # crude-ln slope for bf16 bits
C0 = (127.0 - 0.0430357) * LN2   # bits-line intercept, mu centers the
                                 # f - log2(1+f) ripple at +-0.0298
SLOT = 512                # psum slot stride (f32 elems per partition)

_nc_cache = {}


def _build_nc():
    if 'nc' in _nc_cache:
        return _nc_cache['nc']
    import concourse.bacc as bacc
    import concourse.mybir as mybir
    from concourse.ap import AP
    from concourse.bass import IndirectOffsetOnAxis
    from concourse.tile import TileContext
    from concourse.masks import make_identity

    f32 = mybir.dt.float32
    bf16 = mybir.dt.bfloat16
    i16 = mybir.dt.int16
    i32 = mybir.dt.int32
    AF = mybir.ActivationFunctionType
    ALU = mybir.AluOpType

    nc = bacc.Bacc("TRN2", target_bir_lowering=False, debug=False)
    x_idx = nc.dram_tensor("x_idx", [128, 16], i32, kind="ExternalInput")
    emb = nc.dram_tensor("emb", [V, E], f32, kind="ExternalInput")
    wbd = nc.dram_tensor("wbd", [80, 128], bf16, kind="ExternalInput")
    biasd = nc.dram_tensor("biasd", [128, 1], f32, kind="ExternalInput")
    wout = nc.dram_tensor("wout", [KP, V], bf16, kind="ExternalInput")
    out_b = nc.dram_tensor("out_b", [NR, V], bf16, kind="ExternalOutput")

    with TileContext(nc) as tc:
        with (
            tc.tile_pool(name="const", bufs=1) as cpool,
            tc.tile_pool(name="big", bufs=1, space="PSUM") as bigpool,
            tc.tile_pool(name="gat", bufs=2) as gpool,
            tc.tile_pool(name="scan", bufs=3) as scpool,
            tc.tile_pool(name="ex", bufs=22) as expool,
            tc.tile_pool(name="ob", bufs=3) as obpool,
            tc.tile_pool(name="proj", bufs=3) as prpool,
        ):
            # ---- constants / persistent buffers ----
            wbd_sb = cpool.tile([80, 128], bf16, tag="wbd")
            nc.sync.dma_start(wbd_sb[:, :], wbd[:, :])
            bias_sb = cpool.tile([128, 1], f32, tag="bias")
            nc.sync.dma_start(bias_sb[:, :], biasd[:, :])
            wout_sb = cpool.tile([KP, V], bf16, tag="wout")
            nc.sync.dma_start(wout_sb[:, :], wout[:, :])
            idx_sb = cpool.tile([128, 16], i32, tag="idx")
            nc.sync.dma_start(idx_sb[:, :], x_idx[:, :])
            ident = cpool.tile([128, 128], f32, tag="ident")
            make_identity(nc, ident[:, :])
            czero = cpool.tile([16, BL], f32, tag="czero")
            nc.vector.memset(czero[:, :], 0.0)
            half = cpool.tile([16, 1], f32, tag="half")
            nc.vector.memset(half[:, :], 0.5)
            e_both = cpool.tile([80, NR], bf16, tag="eboth")
            nc.vector.memset(e_both[64:80, 0:BL], 0.0)   # h1[0]=h2[127]=0

            # all 8 PSUM banks, slots = 500 cols at 512-elem stride
            ps = bigpool.tile([128, 4096], f32, tag="ps")

            def slotap(s, n=1):
                if n == 1:
                    return ps[:, SLOT * s:SLOT * s + VT]
                return AP(ps.tensor, ps.offset + SLOT * s,
                          [[4096, 128], [SLOT, n], [1, VT]])

            # ---- embedding gather + transpose into e_both (bf16); emitted
            # just-in-time per 16-step chunk so it overlaps the scan ----
            def emit_gather(c):
                for d in range(2):
                    g = gpool.tile([128, E], f32, tag="g")
                    nc.gpsimd.indirect_dma_start(
                        g[:, :], None, emb[:, :],
                        IndirectOffsetOnAxis(ap=idx_sb[:, 8 * d + c:8 * d + c + 1], axis=0),
                    )
                    pt = ps[0:E, 3584:3712]
                    nc.tensor.transpose(pt, g[:, :], ident[:, :])
                    nc.vector.tensor_copy(
                        e_both[32 * d:32 * d + 32, 128 * c:128 * c + 128], pt)

            # ---- LSTM scan ----
            # gate cols: f@0-15, i@32-47, o@64-79, C@96-111 (fwd8+bwd8 each)
            # gate preacts live in the last 8 cols of bank 7, past slot 7's
            # 500-col tile, so the scan never blocks a 4-slot exp window
            pg = ps[:, 4084:4084 + BL]
            gather_done = [0]

            def emit_scan_step(k):
                want = min((k + 24) // 16 + 1, 8)
                while gather_done[0] < want:
                    emit_gather(gather_done[0])
                    gather_done[0] += 1
                cs = slice(k * BL, (k + 1) * BL)
                nc.tensor.matmul(pg, wbd_sb[:, :], e_both[:, cs],
                                 start=True, stop=True)
                tg = scpool.tile([112, BL], f32, tag="tg")
                nc.scalar.activation(tg[:, :], pg[0:112, :], AF.Tanh,
                                     bias=bias_sb[0:112, 0:1])
                cprev = emit_scan_step.cprev if k > 0 else czero
                u1 = scpool.tile([48, BL], f32, tag="u1")
                nc.vector.scalar_tensor_tensor(u1[32:48, :], tg[0:16, :], 1.0,
                                               cprev[0:16, :], op0=ALU.add,
                                               op1=ALU.mult)
                u2 = scpool.tile([112, BL], f32, tag="u2")
                nc.vector.tensor_tensor(u2[96:112, :], u1[32:48, :], tg[32:48, :],
                                        op=ALU.add)
                cnp = scpool.tile([16, BL], f32, tag="cnp")
                nc.vector.scalar_tensor_tensor(cnp[:, :], u2[96:112, :], 0.5,
                                               tg[96:112, :], op0=ALU.mult,
                                               op1=ALU.add)
                # cnew/w on gpsimd: off the DVE critical path
                cnew = scpool.tile([16, BL], f32, tag="cnew")
                nc.gpsimd.tensor_scalar(cnew[:, :], cnp[:, :], 0.5, None,
                                        op0=ALU.add)
                emit_scan_step.cprev = cnew
                w = scpool.tile([80, BL], f32, tag="w")
                nc.gpsimd.tensor_scalar(w[64:80, :], tg[64:80, :], 0.5, 0.5,
                                        op0=ALU.mult, op1=ALU.add)
                tht = scpool.tile([80, BL], f32, tag="tht")
                nc.scalar.activation(tht[64:80, :], cnp[:, :], AF.Tanh,
                                     bias=half[:, 0:1])
                if k < T - 1:
                    ns = slice((k + 1) * BL, (k + 2) * BL)
                    nc.vector.tensor_tensor(e_both[64:80, ns], w[64:80, :],
                                            tht[64:80, :], op=ALU.mult)

            # ---- projection ----
            ex_of = {}

            def emit_P1(j, wide):
                hb = prpool.tile([KP, 128], bf16, tag="hb")
                # base-8 accesses are illegal, so layer base-0 writes:
                # zeros everywhere, ones over 0:16 (rows 9-15 hit zero wout
                # rows), h1 over 0:8. Row 8 ends up = 1.0 (bias feature).
                nc.vector.memset(hb[0:KP, :], 0.0)
                nc.vector.memset(hb[0:16, :], 1.0)
                nc.vector.tensor_copy(hb[0:8, :],
                                      e_both[64:72, 128 * j:128 * (j + 1)])
                # h2 reversed over t: blocks (127-t)*8, t ascending. Copy
                # the whole 16-row h block (base 64, legal) into hb rows
                # 32:48 (base 32): reversed h1 lands in 32:40 where the
                # wout rows are zero, reversed h2 in 40:48 where they live.
                rev = AP(e_both.tensor,
                         e_both.offset + 64 * NR + (127 - 16 * j) * BL,
                         [[NR, 16], [-BL, 16], [1, BL]])
                nc.vector.tensor_copy(
                    AP(hb.tensor, hb.offset + 32 * 128, [[128, 16], [BL, 16], [1, BL]]),
                    rev)
                # row sums: DVE reduce (1x but cheap-ish) for 2/3 of the
                # windows, GpSimd accumulate for 1/3 to keep DVE headroom
                acc = prpool.tile([128, 4 * VT], f32, tag="acc", bufs=1)
                nc.gpsimd.memset(acc[:, :], 0.0)
                sums = prpool.tile([128, 24], f32, tag="sums")
                nsum = 0
                tiles = []
                ex_of[j] = tiles
                v = 0
                wsel = 0
                widx = 0
                while v < NTILE:
                    n = min(4, NTILE - v)
                    base = 0 if wsel == 0 else 4
                    wsel ^= 1
                    for q in range(n):
                        nc.tensor.matmul(
                            slotap(base + q), hb[:, :],
                            wout_sb[:, (v + q) * VT:(v + q + 1) * VT],
                            start=True, stop=True)
                    ex = expool.tile([128, 4 * VT], bf16, tag="ex")
                    exap = AP(ex.tensor, ex.offset, [[4 * VT, 128], [VT, n], [1, VT]])
                    nc.scalar.activation(exap, slotap(base, n), AF.Exp)
                    if widx % 3 == 2 and v < NTILE - 12:
                        nc.gpsimd.tensor_tensor(
                            acc[:, 0:n * VT], acc[:, 0:n * VT], ex[:, 0:n * VT],
                            op=ALU.add)
                    else:
                        nc.vector.reduce_sum(sums[:, nsum:nsum + 1],
                                             ex[:, 0:n * VT],
                                             axis=mybir.AxisListType.X)
                        nsum += 1
                    tiles.append((ex, v, n))
                    v += n
                    widx += 1
                    yield
                nc.vector.reduce_sum(sums[:, nsum:nsum + 1], acc[:, :],
                                     axis=mybir.AxisListType.X)
                nsum += 1
                red = prpool.tile([128, 4], f32, tag="red")
                nc.vector.reduce_sum(red[:, 0:1], sums[:, 0:nsum],
                                     axis=mybir.AxisListType.X)
                red_of[j] = red

            def emit_L(j, red):
                # lse = ln(red): exponent-bits guess + two Newton steps
                # L += red*exp(-L) - 1 (exp stays in the loaded table set)
                lse = prpool.tile([128, 4], f32, tag="lse")
                nc.vector.tensor_copy(red[:, 1:2], red[:, 0:1].bitcast(i32))
                nc.vector.tensor_scalar(lse[:, 0:1], red[:, 1:2],
                                        LN2 / (1 << 23), -C0,
                                        op0=ALU.mult, op1=ALU.add)
                cur, nxt = 0, 2
                for _ in range(2):
                    e = prpool.tile([128, 1], f32, tag="nwt")
                    nc.scalar.activation(e[:, :], lse[:, cur:cur + 1], AF.Exp,
                                         scale=-1.0)
                    p = prpool.tile([128, 1], f32, tag="nwp")
                    nc.vector.tensor_tensor(p[:, :], e[:, :], red[:, 0:1], op=ALU.mult)
                    nc.vector.scalar_tensor_tensor(lse[:, nxt:nxt + 1], p[:, :], -1.0,
                                                   lse[:, cur:cur + 1], op0=ALU.add,
                                                   op1=ALU.add)
                    cur, nxt = nxt, cur
                # s1 = -C0 - lse : the bias of the crude-ln decode
                s1 = prpool.tile([128, 1], f32, tag="s1")
                nc.vector.tensor_scalar(s1[:, :], lse[:, cur:cur + 1], -1.0, -C0,
                                        op0=ALU.mult, op1=ALU.add)
                return s1

            def emit_P2(j, s1):
                # out = ln(ex) - lse ~= bits16(ex)*KLN + (-C0 - lse);
                # two windows share one staging tile -> one DMA per pair
                row0 = 128 * j
                pend = None
                for ex, v0, n in ex_of.pop(j):
                    if pend is None:
                        ob = obpool.tile([128, 8 * VT], bf16, tag="ob")
                        off = 0
                        pend = (ob, v0)
                    nc.vector.tensor_scalar(ob[:, off:off + n * VT],
                                            ex[:, 0:n * VT].bitcast(i16),
                                            KLN, s1[:, 0:1],
                                            op0=ALU.mult, op1=ALU.add)
                    off += n * VT
                    if off >= 7 * VT:
                        nc.sync.dma_start(
                            out_b[row0:row0 + 128,
                                  pend[1] * VT:pend[1] * VT + off],
                            ob[:, 0:off])
                        pend = None
                    yield
                if pend is not None:
                    nc.sync.dma_start(
                        out_b[row0:row0 + 128, pend[1] * VT:pend[1] * VT + off],
                        ob[:, 0:off])

            def drain(g):
                if g is not None:
                    for _ in g:
                        pass

            # ---- interleaved emission, middle-out slab order. P2(prev)
            # windows interleave with P1(j) windows so each P1 exp's ex-pool
            # allocation follows a P2 drain (no buffer starvation stalls) ----
            order = [3, 4, 2, 5, 1, 6, 0, 7]
            ready = {j: max(16 * j + 15, 127 - 16 * j) for j in range(NSLAB)}
            scan_done = 0
            red_of = {}
            g2 = None
            for idx, j in enumerate(order):
                while scan_done < ready[j]:
                    emit_scan_step(scan_done)
                    scan_done += 1
                g1 = emit_P1(j, scan_done >= T - 1)
                if idx >= 1:
                    pj = order[idx - 1]
                    s1 = emit_L(pj, red_of.pop(pj))
                    g2 = emit_P2(pj, s1)
                more = True
                while more:
                    more = next(g1, -1) != -1
                    if g2 is not None and next(g2, -1) == -1:
                        g2 = None
                drain(g2)
                g2 = None
            while scan_done < T - 1:
                emit_scan_step(scan_done)
                scan_done += 1
            pj = order[-1]
            s1 = emit_L(pj, red_of.pop(pj))
            drain(emit_P2(pj, s1))

    nc.finalize()
    _nc_cache['nc'] = nc
    return nc


def _host_prep(inputs):
    """Per-core input maps: weight layout prep + index sharding."""
    import ml_dtypes
    inp = {k: np.asarray(v) for k, v in inputs.items()}
    # W_bd [80, 128]: rows e1 0-31 | e2 32-63 | h1 64-71 | h2 72-79;
    # cols f@0-15, i@32-47, o@64-79, C@96-111 (fwd 8 then bwd 8 in each
    # block). f/i/o scaled by 0.5 for the tanh-based sigmoid.
    W_bd = np.zeros((80, 128), np.float32)
    bias = np.zeros((128, 1), np.float32)
    for d in range(2):
        sfx = str(d + 1)
        Wf, bf = inp['Wf' + sfx], inp['bf' + sfx]
        Wi, bi = inp['Wi' + sfx], inp['bi' + sfx]
        WC, bC = inp['WC' + sfx], inp['bC' + sfx]
        Wo, bo = inp['Wo' + sfx], inp['bo' + sfx]
        er = slice(d * 32, d * 32 + 32)
        hr = slice(64 + 8 * d, 64 + 8 * d + 8)
        for base, Wg, bg in ((0, Wf, bf), (32, Wi, bi), (64, Wo, bo)):
            cols = slice(base + 8 * d, base + 8 * d + 8)
            W_bd[er, cols] = 0.5 * np.repeat(Wg[8:40].astype(np.float32), 8, axis=1)
            W_bd[hr, cols] = 0.5 * np.repeat(Wg[0:8].astype(np.float32), 8, axis=1)
            bias[cols, 0] = 0.5 * bg[0]
        cc = slice(96 + 8 * d, 96 + 8 * d + 8)
        W_bd[er, cc] = WC[8:40]
        W_bd[hr, cc] = WC[0:8]
        bias[cc, 0] = bC
    # wout48: rows 0-7 Wout[0:8] (h1), 8 bout, 40-47 Wout[8:16] (h2)
    wout40 = np.zeros((KP, V), np.float32)
    wout40[0:8] = inp['Wout'][0:8]
    wout40[8] = inp['bout']
    wout40[40:48] = inp['Wout'][8:16]
    wout40 = wout40.astype(ml_dtypes.bfloat16)
    W_bd = W_bd.astype(ml_dtypes.bfloat16)
    emb = np.ascontiguousarray(inp['emb'].astype(np.float32))
    x = inp['x']
    in_maps = []
    for c in range(NCORES):
        xl = x[:, c * BL:(c + 1) * BL].astype(np.int32)        # [T, BL]
        fwd = xl.reshape(-1)
        rev = xl[::-1].reshape(-1)
        xi = np.concatenate([fwd.reshape(8, 128).T, rev.reshape(8, 128).T],
                            axis=1)                            # [128, 16]
        in_maps.append({
            "x_idx": np.ascontiguousarray(xi),
            "emb": emb,
            "wbd": np.ascontiguousarray(W_bd),
            "biasd": bias,
            "wout": np.ascontiguousarray(wout40),
        })
    return in_maps


def kernel(**inputs):
    from concourse.bass_utils import run_bass_kernel_spmd
    nc = _build_nc()
    in_maps = _host_prep(inputs)
    res = run_bass_kernel_spmd(nc, in_maps, list(range(NCORES)))
    out = np.empty((T, B, V), np.float32)
    for c in range(NCORES):
        out[:, c * BL:(c + 1) * BL, :] = (
            res.results[c]["out_b"].astype(np.float32).reshape(T, BL, V))
    return out
